# revision 8
# baseline (speedup 1.0000x reference)
"""Trainium2 Bass kernel for nn_CompressedInteractionNet_31997506355236.

Reference math (per batch b, channel k, dim d; m == H == 64, D == 16, vk == 16):
    x0r[b,d,:]  = x_0[b,:,d]                      # [m]
    xhr[b,d,:]  = x_0[b].reshape(D, H)[d]         # [H] (flat reinterpretation)
    out[b,k,d]  = sum_v (x0r[b,d] @ Vm[k,0,:,v]) * (Vh[k,0,v,:] @ xhr[b,d])

Strategy: pure data parallel over B across 8 cores (sharding hint). Per core:
    A  [bd, (k,v)] = X0T.T @ VmF      (PE, f32r)
    Bt [bd, (k,v)] = XhrT.T @ VhF     (PE, f32r; VhF/XhrT built via PE transposes)
    P = A * Bt                        (DVE)
    O[bd, k] = sum_v P[bd, k, v]      (GPSIMD half-add + DVE reduce)
    out = O.T                         (PE transpose, ACT copyback, DMA)
"""

import numpy as np

import concourse.bass as bass
import concourse.tile as tile
from concourse import bacc, mybir
from concourse.bass_utils import run_bass_kernel_spmd
from concourse.masks import make_identity

# Problem constants (hardcoded; kernel must be self-contained).
B, M, D = 128, 64, 16
HK, VK = 64, 16
H = 64
NCORES = 8
BL = B // NCORES          # batch per core = 16
BD = BL * D               # row count per core = 256
KV = HK * VK              # 1024
F32 = mybir.dt.float32
F32R = mybir.dt.float32r

_CACHE = {}


def _r(ap):
    return ap.bitcast(F32R)


def build_bass():
    nc = bacc.Bacc("TRN2", target_bir_lowering=False, debug=False,
                   num_devices=NCORES)

    x0 = nc.dram_tensor("x0", [BL, M, D], F32, kind="ExternalInput")
    vm = nc.dram_tensor("vm", [HK, M, VK], F32, kind="ExternalInput")
    vh = nc.dram_tensor("vh", [HK, VK, H], F32, kind="ExternalInput")
    out = nc.dram_tensor("out", [BL, HK, D], F32, kind="ExternalOutput")

    with tile.TileContext(nc) as tc:
        with (
            tc.tile_pool(name="const", bufs=1) as const,
            tc.tile_pool(name="w", bufs=1) as w,
            tc.tile_pool(name="work", bufs=2) as work,
            tc.tile_pool(name="pt", bufs=1, space="PSUM") as pt,
            tc.tile_pool(name="pab", bufs=1, space="PSUM") as pab,
        )        :
            ident_f = const.tile([128, 128], F32)
            make_identity(nc, ident_f)
            ident_r = const.tile([128, 128], F32R)
            nc.vector.tensor_copy(ident_r[:], ident_f[:])

            # ---- loads -------------------------------------------------
            # X0T[i, (b,d)] = x0[b, i, d]   (64B runs)
            x0t = w.tile([M, BL, D], F32R)
            nc.gpsimd.dma_start(x0t[:],
                                x0.ap().bitcast(F32R).rearrange("b m d -> m b d"))

            # VmF[i, (k,v)] = vm[k, i, v]   (64B runs); split for 2 queues
            vmf = w.tile([M, HK, VK], F32R)
            vm_r = vm.ap().bitcast(F32R)
            nc.sync.dma_start(vmf[:, 0:32, :],
                              vm_r[0:32].rearrange("k i v -> i k v"))
            nc.sync.dma_start(vmf[:, 32:64, :],
                              vm_r[32:64].rearrange("k i v -> i k v"))

            # Vh natural rows: [(k,v), j] -> tiles [128, 8, 64] (256B runs)
            vh_nat = w.tile([128, 8, H], F32R)
            nc.sync.dma_start(
                vh_nat[:],
                vh.ap().bitcast(F32R).rearrange("k v j -> (k v) j").rearrange(
                    "(t p) j -> p t j", p=128),
            )

            # Xhr[(b,dn), j] = x0 flat per-b block [dn*64 + j]  (256B runs)
            xhr_nat = w.tile([128, 2, H], F32R)
            xhr_view = x0.ap().bitcast(F32R).rearrange(
                "b m d -> b (m d)").rearrange("b (dn j) -> (b dn) j", j=H)
            nc.scalar.dma_start(xhr_nat[:, 0, :], xhr_view[0:128])
            nc.scalar.dma_start(xhr_nat[:, 1, :], xhr_view[128:256])

            # ---- on-chip transposes (PE) --------------------------------
            # XhrT [j, bd]
            p_xhr = pt.tile([H, 2, 128], F32R, tag="p_xhr")
            for t in range(2):
                nc.tensor.transpose(p_xhr[:, t, :], xhr_nat[:, t, :], ident_r)
            xhrt = w.tile([H, 2, 128], F32R)
            nc.scalar.copy(xhrt[:], p_xhr[:])

            # VhF [j, (k,v)]
            p_vh = pt.tile([H, 8, 128], F32R, tag="p_vh")
            for t in range(8):
                nc.tensor.transpose(p_vh[:, t, :], vh_nat[:, t, :], ident_r)
            vhf = w.tile([H, KV], F32R)
            nc.scalar.copy(vhf[:, 0:512], p_vh[:, 0:4, :])
            nc.scalar.copy(vhf[:, 512:1024], p_vh[:, 4:8, :])

            # ---- products + epilogue per (128-row chunk, kv half) -------
            ot_sb = w.tile([HK, 2, 128], F32)  # O^T staging, both chunks
            vmf_flat = vmf.rearrange("i k v -> i (k v)")
            for c in range(2):
                lhs_a = x0t[:, 8 * c:8 * (c + 1), :]     # [64, 128]
                lhs_b = xhrt[:, c, :]                     # [64, 128]
                o_sb = work.tile([128, HK], F32, tag="o_sb")
                for h in range(2):
                    sl = slice(512 * h, 512 * (h + 1))
                    psum_a = pab.tile([128, 512], F32, tag="a")
                    psum_b = pab.tile([128, 512], F32, tag="b")
                    nc.tensor.matmul(psum_a[:], lhs_a, vmf_flat[:, sl],
                                     start=True, stop=True)
                    nc.tensor.matmul(psum_b[:], lhs_b, vhf[:, sl],
                                     start=True, stop=True)

                    # B -> SBUF (ACT), P = A * B (DVE), then sum over v.
                    b_sb = work.tile([128, 32, VK], F32, tag="b_sb")
                    nc.scalar.copy(b_sb.rearrange("p k v -> p (k v)"),
                                   psum_b[:])
                    p_sb = work.tile([128, 32, VK], F32, tag="p_sb")
                    nc.vector.tensor_mul(
                        out=p_sb.rearrange("p k v -> p (k v)"),
                        in0=psum_a[:],
                        in1=b_sb.rearrange("p k v -> p (k v)"))
                    t1 = work.tile([128, 32, VK // 2], F32, tag="t1")
                    nc.gpsimd.tensor_tensor(t1[:], p_sb[:, :, 0:8],
                                            p_sb[:, :, 8:16],
                                            mybir.AluOpType.add)
                    nc.vector.tensor_reduce(out=o_sb[:, 32 * h:32 * (h + 1)],
                                            in_=t1[:],
                                            axis=mybir.AxisListType.X,
                                            op=mybir.AluOpType.add)

                # O^T chunk
                p_o = pt.tile([HK, 128], F32, tag="p_o")
                nc.tensor.transpose(p_o[:], o_sb[:], ident_f)
                nc.scalar.copy(ot_sb[:, c, :], p_o[:])

            # out[b,k,d] <- ot_sb[k, (b,d)]
            nc.sync.dma_start(
                out.ap().rearrange("b k d -> k b d"),
                ot_sb.rearrange("k c bd -> k (c bd)").rearrange(
                    "k (b d) -> k b d", d=D),
            )

    nc.compile()
    return nc


def run(x_0, x_h, Vm, Vh, **spmd_kwargs):
    x_0 = np.ascontiguousarray(np.asarray(x_0), dtype=np.float32)
    vm = np.ascontiguousarray(np.asarray(Vm)[:, 0], dtype=np.float32)
    vh = np.ascontiguousarray(np.asarray(Vh)[:, 0], dtype=np.float32)

    if "nc" not in _CACHE:
        _CACHE["nc"] = build_bass()
    nc = _CACHE["nc"]

    in_maps = [
        {"x0": x_0[BL * c:BL * (c + 1)], "vm": vm, "vh": vh}
        for c in range(NCORES)
    ]
    res = run_bass_kernel_spmd(nc, in_maps, core_ids=list(range(NCORES)),
                               **spmd_kwargs)
    out = np.concatenate([res.results[c]["out"] for c in range(NCORES)], axis=0)
    return out, res


def kernel(x_0, x_h, Vm, Vh):
    return run(x_0, x_h, Vm, Vh)[0]


if __name__ == "__main__":
    rng = np.random.default_rng(0)
    x_0 = rng.standard_normal((B, M, D)).astype(np.float32)
    x_h = rng.standard_normal((B, H, D)).astype(np.float32)
    Vm = rng.standard_normal((HK, 1, M, VK)).astype(np.float32)
    Vh = rng.standard_normal((HK, 1, VK, H)).astype(np.float32)
    got = kernel(x_0, x_h, Vm, Vh)

    x0r = np.transpose(x_0, (0, 2, 1))
    xhr = x_0.reshape(B, D, H)
    a = np.einsum("bdi,kiv->bkdv", x0r, Vm[:, 0])
    bb = np.einsum("bdj,kvj->bkdv", xhr, Vh[:, 0])
    want = np.einsum("bkdv,bkdv->bkd", a, bb)
    err = np.abs(got - want).max() / np.abs(want).max()
    print("rel err:", err)


# revision 11
# speedup vs baseline: 1.1683x; 1.1683x over previous
"""Trainium2 Bass kernel for nn_CompressedInteractionNet_31997506355236.

Reference math (per batch b, channel k, dim d; m == H == 64, D == 16, vk == 16):
    x0r[b,d,:]  = x_0[b,:,d]                      # [m]
    xhr[b,d,:]  = x_0[b].reshape(D, H)[d]         # [H] (flat reinterpretation)
    out[b,k,d]  = sum_v (x0r[b,d] @ Vm[k,0,:,v]) * (Vh[k,0,v,:] @ xhr[b,d])

Strategy: pure data parallel over B across 8 cores. Per core (16 batches,
bd = 256 rows):
    A  [bd, (k,v)] = X0T.T @ VmF      (PE, f32r)
    Bt [bd, (k,v)] = XhrT.T @ VhF     (PE, f32r; VhF/XhrT built via PE transposes
                                       since j is innermost in DRAM for both)
    P = A * Bt                        (DVE; one PSUM + one SBUF operand)
    O[bd, k] = sum_v P[bd, k, v]      (DVE reduce over innermost 16)
    out = O.T                         (PE transpose; DMA straight from PSUM)
"""

import numpy as np

import concourse.bass as bass
import concourse.tile as tile
from concourse import bacc, mybir
from concourse.bass_utils import run_bass_kernel_spmd

# Problem constants (hardcoded; kernel must be self-contained).
B, M, D = 128, 64, 16
HK, VK = 64, 16
H = 64
NCORES = 8
BL = B // NCORES          # batch per core = 16
BD = BL * D               # row count per core = 256
KV = HK * VK              # 1024
F32 = mybir.dt.float32
F32R = mybir.dt.float32r

_CACHE = {}


def build_bass():
    nc = bacc.Bacc("TRN2", target_bir_lowering=False, debug=False,
                   num_devices=NCORES, enable_partition_id=False)

    x0 = nc.dram_tensor("x0", [BL, M, D], F32, kind="ExternalInput")
    vm = nc.dram_tensor("vm", [HK, M, VK], F32, kind="ExternalInput")
    vh = nc.dram_tensor("vh", [HK, VK, H], F32, kind="ExternalInput")
    out = nc.dram_tensor("out", [BL, HK, D], F32, kind="ExternalOutput")

    vm_r = vm.ap().bitcast(F32R)
    vh_r = vh.ap().bitcast(F32R)
    x0_r = x0.ap().bitcast(F32R)
    # Vh natural rows [(k,v), j]; halves align with kv halves of products.
    vh_rows = vh_r.rearrange("k v j -> (k v) j")
    # Xhr[(b,dn), j]: per-b flat block, row dn covers elements dn*64..dn*64+63.
    xhr_view = x0_r.rearrange("b m d -> b (m d)").rearrange(
        "b (dn j) -> (b dn) j", j=H)

    with tile.TileContext(nc) as tc:
        with (
            tc.tile_pool(name="const", bufs=1) as const,
            tc.tile_pool(name="w", bufs=1) as w,
            tc.tile_pool(name="work", bufs=3) as work,
            tc.tile_pool(name="ptw", bufs=2, space="PSUM") as ptw,
            tc.tile_pool(name="pts", bufs=1, space="PSUM") as pts,
            tc.tile_pool(name="pab", bufs=2, space="PSUM") as pab,
        ):
            # identity for PE transposes (f32 built on DVE, rounded to f32r)
            ident_f = const.tile([128, 128], F32)
            nc.gpsimd.memset(ident_f, 0.0)
            nc.gpsimd.affine_select(
                out=ident_f, in_=ident_f,
                compare_op=mybir.AluOpType.not_equal, fill=1.0, base=0,
                pattern=[[-1, 128]], channel_multiplier=1)
            ident_r = const.tile([128, 128], F32R)
            nc.vector.tensor_copy(ident_r[:], ident_f[:])

            # ---- loads --------------------------------------------------
            # SP queue: xhr0, vh half0, vm half0   (each needed first)
            # ACT queue: xhr1, vh half1, vm half1
            # GPSIMD (SWDGE): x0t
            xhr_nat = w.tile([128, 2, H], F32R)
            nc.sync.dma_start(xhr_nat[:, 0, :], xhr_view[0:128])
            nc.scalar.dma_start(xhr_nat[:, 1, :], xhr_view[128:256])

            vh_nat = w.tile([128, 8, H], F32R)
            nc.sync.dma_start(
                vh_nat[:, 0:4, :],
                vh_rows[0:512].rearrange("(t p) j -> p t j", p=128))
            nc.scalar.dma_start(
                vh_nat[:, 4:8, :],
                vh_rows[512:1024].rearrange("(t p) j -> p t j", p=128))

            vmf = w.tile([M, HK, VK], F32R)
            nc.sync.dma_start(vmf[:, 0:32, :],
                              vm_r[0:32].rearrange("k i v -> i k v"))
            nc.scalar.dma_start(vmf[:, 32:64, :],
                              vm_r[32:64].rearrange("k i v -> i k v"))
            vmf_flat = vmf.rearrange("i k v -> i (k v)")

            x0t = w.tile([M, BL, D], F32R)
            nc.gpsimd.dma_start(x0t[:], x0_r.rearrange("b m d -> m b d"))

            # ---- on-chip transposes (PE; early to warm the PE clock) ----
            # XhrT [j, bd]
            p_xhr = pts.tile([H, 2, 128], F32R, tag="small")
            for t in range(2):
                nc.tensor.transpose(p_xhr[:, t, :], xhr_nat[:, t, :], ident_r)
            xhrt = w.tile([H, 2, 128], F32R)
            nc.scalar.copy(xhrt[:], p_xhr[:])

            # VhF [j, (k,v)] in two waves of 4 (each wave = one kv half)
            vhf = w.tile([H, KV], F32R)
            for wv in range(2):
                p_vh = ptw.tile([H, 4, 128], F32R, tag="wave")
                for t in range(4):
                    nc.tensor.transpose(p_vh[:, t, :],
                                        vh_nat[:, 4 * wv + t, :], ident_r)
                nc.scalar.copy(vhf[:, 512 * wv:512 * (wv + 1)], p_vh[:])

            # ---- products + epilogue per (128-row chunk, kv half) -------
            for c in range(2):
                lhs_a = x0t[:, 8 * c:8 * (c + 1), :]     # [64, 128]
                lhs_b = xhrt[:, c, :]                     # [64, 128]
                o_sb = work.tile([128, HK], F32, tag="o_sb")
                for h in range(2):
                    sl = slice(512 * h, 512 * (h + 1))
                    psum_a = pab.tile([128, 512], F32, tag="a")
                    psum_b = pab.tile([128, 512], F32, tag="b")
                    nc.tensor.matmul(psum_a[:], lhs_a, vmf_flat[:, sl],
                                     start=True, stop=True)
                    nc.tensor.matmul(psum_b[:], lhs_b, vhf[:, sl],
                                     start=True, stop=True)

                    # B -> SBUF (ACT), P = A * B (DVE), sum over v (DVE).
                    b_sb = work.tile([128, 32, VK], F32, tag="b_sb")
                    nc.scalar.copy(b_sb.rearrange("p k v -> p (k v)"),
                                   psum_b[:])
                    p_sb = work.tile([128, 32, VK], F32, tag="p_sb")
                    nc.vector.tensor_mul(
                        out=p_sb.rearrange("p k v -> p (k v)"),
                        in0=psum_a[:],
                        in1=b_sb.rearrange("p k v -> p (k v)"))
                    nc.vector.tensor_reduce(out=o_sb[:, 32 * h:32 * (h + 1)],
                                            in_=p_sb[:],
                                            axis=mybir.AxisListType.X,
                                            op=mybir.AluOpType.add)

                # O^T chunk -> SBUF -> DMA out
                p_o = pts.tile([HK, 128], F32, tag="small")
                nc.tensor.transpose(p_o[:], o_sb[:], ident_f)
                ot_sb = work.tile([HK, 128], F32, tag="ot_sb")
                nc.scalar.copy(ot_sb[:], p_o[:])
                nc.sync.dma_start(
                    out.ap()[8 * c:8 * (c + 1)].rearrange("b k d -> k b d"),
                    ot_sb.rearrange("k (b d) -> k b d", d=D))

    nc.compile()
    return nc


def run(x_0, x_h, Vm, Vh, **spmd_kwargs):
    x_0 = np.ascontiguousarray(np.asarray(x_0), dtype=np.float32)
    vm = np.ascontiguousarray(np.asarray(Vm)[:, 0], dtype=np.float32)
    vh = np.ascontiguousarray(np.asarray(Vh)[:, 0], dtype=np.float32)

    if "nc" not in _CACHE:
        _CACHE["nc"] = build_bass()
    nc = _CACHE["nc"]

    in_maps = [
        {"x0": x_0[BL * c:BL * (c + 1)], "vm": vm, "vh": vh}
        for c in range(NCORES)
    ]
    res = run_bass_kernel_spmd(nc, in_maps, core_ids=list(range(NCORES)),
                               **spmd_kwargs)
    out = np.concatenate([res.results[c]["out"] for c in range(NCORES)], axis=0)
    return out, res


def kernel(x_0, x_h, Vm, Vh):
    return run(x_0, x_h, Vm, Vh)[0]


if __name__ == "__main__":
    rng = np.random.default_rng(0)
    x_0 = rng.standard_normal((B, M, D)).astype(np.float32)
    x_h = rng.standard_normal((B, H, D)).astype(np.float32)
    Vm = rng.standard_normal((HK, 1, M, VK)).astype(np.float32)
    Vh = rng.standard_normal((HK, 1, VK, H)).astype(np.float32)
    got = kernel(x_0, x_h, Vm, Vh)

    x0r = np.transpose(x_0, (0, 2, 1))
    xhr = x_0.reshape(B, D, H)
    a = np.einsum("bdi,kiv->bkdv", x0r, Vm[:, 0])
    bb = np.einsum("bdj,kvj->bkdv", xhr, Vh[:, 0])
    want = np.einsum("bkdv,bkdv->bkd", a, bb)
    err = np.abs(got - want).max() / np.abs(want).max()
    print("rel err:", err)
